# revision 5
# baseline (speedup 1.0000x reference)
"""GroupedQueryAttention Trainium2 kernel (8 NeuronCores).

Sharding: core c -> (batch b = c//4, kv-group g = c%4).
Each core computes its group's 4 query heads over its batch, then the
four cores of a batch AllGather ctx^T (per head) and each computes a
512-column slice of the output projection (tensor-parallel along d_out).

Layout trick: scores are computed transposed (S^T[k, q]) so that
A^T = exp(S^T) is directly the lhsT of the ctx matmul (contract over k)
with no transposes of the big attention matrix. The softmax denominator
comes for free as a 129th "ones" column appended to V. Normalization is
a per-partition scale of ctx[q, :] after the ctx matmul.

All matmul operands are bf16 (fp32 PSUM accumulation); measured end-to-end
max rel err vs the fp32 reference ~4e-3.
"""

from contextlib import ExitStack

import numpy as np
import ml_dtypes

import concourse.bass as bass
import concourse.bacc as bacc
import concourse.tile as tile
from concourse import mybir
from concourse.bass_utils import run_bass_kernel_spmd
from concourse.masks import make_identity
from concourse.tile_rust import add_dep_helper

BF16 = mybir.dt.bfloat16
F32 = mybir.dt.float32

B = 2
S = 2048
D = 2048
G = 4  # kv groups
HPG = 4  # heads per group
HD = 128  # head dim
QC = 512  # q-chunk (columns per S^T block / projection chunk)
NQC = S // QC  # 4
NKT = S // 128  # 16 k-tiles
NDC = D // 128  # 16 d_in chunks
SCALE = 1.0 / float(np.sqrt(HD))
N_CORES = 8
REPLICA_GROUPS = [[0, 1, 2, 3], [4, 5, 6, 7]]


def _build_program(repeat: int = 1):
    nc = bacc.Bacc("TRN2", target_bir_lowering=False, debug=True)

    xq = nc.declare_dram_parameter("xq", [NQC, NDC, 128, QC], BF16, isOutput=False)
    wq = nc.declare_dram_parameter("wq", [NDC, 128, HPG * HD], BF16, isOutput=False)
    wk = nc.declare_dram_parameter("wk", [NDC, 128, HD], BF16, isOutput=False)
    wv = nc.declare_dram_parameter("wv", [NDC, 128, HD], BF16, isOutput=False)
    wo = nc.declare_dram_parameter("wo", [NDC, 128, 512], BF16, isOutput=False)
    bo = nc.declare_dram_parameter("bo", [1, 512], BF16, isOutput=False)
    msk = nc.declare_dram_parameter("msk", [128, 896], BF16, isOutput=False)
    out_ext = nc.declare_dram_parameter("out", [S, 512], F32, isOutput=True)

    # AllGather outputs (Shared address space, one per head per repeat)
    gath_all = [
        [nc.dram_tensor(f"gath{r}_{h}", [G, HD, S], BF16) for h in range(HPG)]
        for r in range(repeat)
    ]

    with tile.TileContext(nc) as tc, ExitStack() as es:
        singles = es.enter_context(tc.tile_pool(name="singles", bufs=1))
        wpool = es.enter_context(tc.tile_pool(name="w", bufs=1))
        xpool = es.enter_context(tc.tile_pool(name="x", bufs=2))
        qkpool = es.enter_context(tc.tile_pool(name="qk", bufs=1))
        apool = es.enter_context(tc.tile_pool(name="a", bufs=32))
        spool = es.enter_context(tc.tile_pool(name="sm", bufs=4))
        cpool = es.enter_context(tc.tile_pool(name="cs", bufs=6))
        ps_big = es.enter_context(tc.tile_pool(name="psb", bufs=4, space="PSUM"))
        ps_small = es.enter_context(tc.tile_pool(name="pss", bufs=4, space="PSUM"))
        dram = es.enter_context(tc.tile_pool(name="dram", bufs=1, space="DRAM"))

        # --- constants ---
        ident = singles.tile([128, 128], BF16, tag="ident")
        make_identity(nc, ident)
        ones1 = singles.tile([1, 128], BF16, tag="ones1")
        nc.vector.memset(ones1, 1.0)
        bo_sb = singles.tile([1, 512], BF16, tag="bo")
        nc.sync.dma_start(out=bo_sb, in_=bo[:, :])
        mask_sb = singles.tile([128, 896], BF16, tag="mask")
        nc.sync.dma_start(out=mask_sb, in_=msk[:, :])

        # --- resident weights (each loaded by ONE batched DMA) ---
        wqall = wpool.tile([128, NDC, HPG * HD], BF16, tag="wqall")
        nc.sync.dma_start(out=wqall, in_=wq.rearrange("a p q -> p a q"))
        wkall = wpool.tile([128, NDC, HD], BF16, tag="wkall")
        nc.sync.dma_start(out=wkall, in_=wk.rearrange("a p q -> p a q"))
        wvall = wpool.tile([128, NDC, HD], BF16, tag="wvall")
        nc.sync.dma_start(out=wvall, in_=wv.rearrange("a p q -> p a q"))
        woall = wpool.tile([128, NDC, 512], BF16, tag="woall")
        nc.sync.dma_start(out=woall, in_=wo.rearrange("a p q -> p a q"))
        wq_sb = [wqall[:, dc, :] for dc in range(NDC)]
        wk_sb = [wkall[:, dc, :] for dc in range(NDC)]
        wv_sb = [wvall[:, dc, :] for dc in range(NDC)]
        wo_sb = [woall[:, dc, :] for dc in range(NDC)]

        # --- persistent activations ---
        qT = [qkpool.tile([128, S], BF16, tag=f"qT{h}", name=f"qT{h}") for h in range(HPG)]
        kT = qkpool.tile([128, S], BF16, tag="kT")
        vext = [
            qkpool.tile([128, HD + 1], BF16, tag=f"v{i}", name=f"v{i}")
            for i in range(NKT)
        ]

        for rep in range(repeat):
            gath = gath_all[rep]

            # ============ Phase 1: projections ============
            for qc in range(NQC):
                xstrip = xpool.tile([128, NDC, QC], BF16, tag="xs")
                nc.sync.dma_start(
                    out=xstrip, in_=xq[qc].rearrange("a p q -> p a q")
                )
                xs = [xstrip[:, dc, :] for dc in range(NDC)]
                # Q^T per head: [dh=128, q 512]
                for h in range(HPG):
                    ps = ps_big.tile([128, QC], F32, tag="big")
                    for dc in range(NDC):
                        nc.tensor.matmul(
                            ps,
                            lhsT=wq_sb[dc][:, h * HD : (h + 1) * HD],
                            rhs=xs[dc],
                            start=(dc == 0),
                            stop=(dc == NDC - 1),
                        )
                    nc.vector.tensor_copy(qT[h][:, qc * QC : (qc + 1) * QC], ps)
                # K^T: [dh, q 512]
                ps = ps_big.tile([128, QC], F32, tag="big")
                for dc in range(NDC):
                    nc.tensor.matmul(
                        ps,
                        lhsT=wk_sb[dc],
                        rhs=xs[dc],
                        start=(dc == 0),
                        stop=(dc == NDC - 1),
                    )
                nc.vector.tensor_copy(kT[:, qc * QC : (qc + 1) * QC], ps)
                # V: [s-tile 128, dv 128] (natural orientation, lhsT = x^T block)
                for st in range(4):
                    kt = qc * 4 + st
                    ps = ps_small.tile([128, HD + 1], F32, tag="small", bufs=2)
                    for dc in range(NDC):
                        nc.tensor.matmul(
                            ps[:, 0:HD],
                            lhsT=xs[dc][:, st * 128 : (st + 1) * 128],
                            rhs=wv_sb[dc],
                            start=(dc == 0),
                            stop=(dc == NDC - 1),
                        )
                    nc.vector.tensor_copy(vext[kt][:, 0:HD], ps[:, 0:HD])
                    nc.vector.memset(vext[kt][:, HD : HD + 1], 1.0)

            # ============ Phase 2: attention per (head, q-chunk) ============
            colls = []
            for h in range(HPG):
                ct_dram = dram.tile([HD, S], BF16, tag=f"ct{h}")
                for qc in range(NQC):
                    nkt = 4 * qc + 4  # causal: k-tiles 0 .. 4qc+3
                    a_blocks = []
                    for kt in range(nkt):
                        ps = ps_big.tile([128, QC], F32, tag="big")
                        nc.tensor.matmul(
                            ps,
                            lhsT=kT[:, kt * 128 : (kt + 1) * 128],
                            rhs=qT[h][:, qc * QC : (qc + 1) * QC],
                            start=True,
                            stop=True,
                        )
                        a = apool.tile([128, QC], BF16, tag="a")
                        nc.scalar.activation(
                            out=a,
                            in_=ps,
                            func=mybir.ActivationFunctionType.Exp,
                            scale=SCALE,
                        )
                        if kt >= 4 * qc:  # diagonal block: causal mask (post-exp)
                            off = 128 * kt - 512 * qc
                            nc.vector.tensor_mul(
                                a, a, mask_sb[:, 384 - off : 384 - off + QC]
                            )
                        a_blocks.append(a)
                    ct = cpool.tile([128, QC], BF16, tag="ct")
                    for st in range(4):
                        qt = qc * 4 + st
                        cps = ps_small.tile([128, HD + 1], F32, tag="small", bufs=2)
                        for kt in range(qt + 1):
                            nc.tensor.matmul(
                                cps,
                                lhsT=a_blocks[kt][:, st * 128 : (st + 1) * 128],
                                rhs=vext[kt],
                                start=(kt == 0),
                                stop=(kt == qt),
                            )
                        zr = cpool.tile([128, 1], F32, tag="zr")
                        nc.vector.reciprocal(zr, cps[:, HD : HD + 1])
                        cs = cpool.tile([128, HD], BF16, tag="cs")
                        nc.vector.tensor_scalar_mul(cs, cps[:, 0:HD], zr)
                        tp = ps_small.tile([128, 128], BF16, tag="tp", bufs=2)
                        nc.tensor.transpose(tp, cs, ident)
                        nc.vector.tensor_copy(ct[:, st * 128 : (st + 1) * 128], tp)
                    nc.sync.dma_start(
                        out=ct_dram[:, qc * QC : (qc + 1) * QC], in_=ct
                    )
                coll = nc.gpsimd.collective_compute(
                    "AllGather",
                    mybir.AluOpType.bypass,
                    replica_groups=REPLICA_GROUPS,
                    ins=[ct_dram[:, :].opt()],
                    outs=[gath[h][:, :, :].opt()],
                )
                colls.append(coll)

            # ============ Phase 3: output projection (columns g*512..) ============
            for ss in range(4):  # s-strips of 512 rows
                ops = []
                for st in range(4):
                    ps = ps_big.tile([128, 512], F32, tag="big")
                    # bias via K=1 matmul: out += ones^T @ bo
                    nc.tensor.matmul(ps, lhsT=ones1, rhs=bo_sb, start=True, stop=False)
                    ops.append(ps)
                for h in range(HPG):
                    cstrip = spool.tile([128, G, QC], BF16, tag="cstrip", bufs=6)
                    d = nc.sync.dma_start(
                        out=cstrip,
                        in_=gath[h][:, :, ss * 512 : (ss + 1) * 512].rearrange(
                            "g p q -> p g q"
                        ),
                    )
                    # shadow-memory tracking of plain (non-pool) DRAM tensors is
                    # uncertain; make the read explicitly wait for the AllGather
                    # that produces gath[h].
                    add_dep_helper(d.ins, colls[h].ins, reason="gather->read")
                    for g in range(G):
                        last = h == HPG - 1 and g == G - 1
                        for st in range(4):
                            nc.tensor.matmul(
                                ops[st],
                                lhsT=cstrip[:, g, st * 128 : (st + 1) * 128],
                                rhs=wo_sb[4 * g + h],
                                start=False,
                                stop=last,
                            )
                for st in range(4):
                    osb = spool.tile([128, 512], F32, tag="osb")
                    nc.vector.tensor_copy(osb, ops[st])
                    nc.sync.dma_start(
                        out=out_ext[ss * 512 + st * 128 : ss * 512 + (st + 1) * 128, :],
                        in_=osb,
                    )

    nc.compile()
    return nc


def _make_mask() -> np.ndarray:
    # base[k, j] = 1.0 if (j - 384) >= k else 0; diag block with offset
    # `off` uses columns [384-off : 896-off]: mask[k, q'] = (q' >= k + off).
    j = np.arange(896)[None, :]
    k = np.arange(128)[:, None]
    return ((j - 384) >= k).astype(ml_dtypes.bfloat16)


def _make_in_maps(inputs) -> list[dict]:
    x = np.asarray(inputs["x"], dtype=np.float32)
    Wq = np.asarray(inputs["Wq"], dtype=np.float32)
    Wk = np.asarray(inputs["Wk"], dtype=np.float32)
    Wv = np.asarray(inputs["Wv"], dtype=np.float32)
    Wo = np.asarray(inputs["Wo"], dtype=np.float32)
    bo = np.asarray(inputs["bo"], dtype=np.float32)

    bf = ml_dtypes.bfloat16
    mask = _make_mask()

    # x^T tiled: [qc, dc, 128, 512] per batch
    xqs = []
    for b in range(B):
        xT = np.ascontiguousarray(x[b].T.astype(bf))  # [d, s]
        xqs.append(
            np.ascontiguousarray(xT.reshape(NDC, 128, NQC, QC).transpose(2, 0, 1, 3))
        )

    in_maps = []
    for c in range(N_CORES):
        b, g = c // 4, c % 4
        in_maps.append(
            {
                "xq": xqs[b],
                "wq": np.ascontiguousarray(
                    Wq[:, g * 512 : (g + 1) * 512].astype(bf).reshape(NDC, 128, 512)
                ),
                "wk": np.ascontiguousarray(
                    Wk[:, g * HD : (g + 1) * HD].astype(bf).reshape(NDC, 128, HD)
                ),
                "wv": np.ascontiguousarray(
                    Wv[:, g * HD : (g + 1) * HD].astype(bf).reshape(NDC, 128, HD)
                ),
                "wo": np.ascontiguousarray(
                    Wo[:, g * 512 : (g + 1) * 512].astype(bf).reshape(NDC, 128, 512)
                ),
                "bo": np.ascontiguousarray(
                    bo[g * 512 : (g + 1) * 512].astype(bf).reshape(1, 512)
                ),
                "msk": mask,
            }
        )
    return in_maps


def _assemble(results) -> np.ndarray:
    out = np.empty((B, S, D), dtype=np.float32)
    for c in range(N_CORES):
        b, g = c // 4, c % 4
        out[b][:, g * 512 : (g + 1) * 512] = results[c]["out"]
    return out


def kernel(**inputs) -> np.ndarray:
    in_maps = _make_in_maps(inputs)
    nc = _build_program()
    res = run_bass_kernel_spmd(nc, in_maps, list(range(N_CORES)))
    return _assemble(res.results)
